# revision 19
# baseline (speedup 1.0000x reference)
"""Soft-kNN imputation kernel for Trainium2 (8 NeuronCores, SPMD).

Problem: for a single query X_missing [64], over X_train [1M, 64]:
  d_i   = ||x_i - q||_2
  w_i   = softmax(-d_i)            (tau = 1.0)
  out   = sum over top-32 w_i * y_train[i]     -> [1, 64]

Memory-bound problem: every train row must enter the softmax denominator
Z and the top-k scan, so the whole shard must cross HBM once. The f32
rows are quantized to fp8(e4m3) on the host, cutting per-core traffic
from 32 MB to ~8.1 MB (~25-27 us at the ~300-330 GB/s sustained HBM
read rate measured on this part). Exactness is recovered two ways:

  - d^2 = ||x~||^2 - 2 x~.q + ||q||^2 with the row norms of the
    *quantized* rows precomputed on the host and shipped as uint8
    against a runtime affine (scale via scalar_tensor_tensor's [128,1]
    scalar, offset folded into the activation bias), so the device-side
    error is fp8 rounding of the cross term (~2e-2 relative per weight,
    zero-mean across rows) + ~1e-2 from the norm quantization. Z (a
    1M-term sum) keeps ~1e-4 accuracy and top-k selection is unaffected
    at margin.
  - the host re-ranks the top ~200 candidates with exact f64 distances,
    so the final 32 weights are exact up to the global Z estimate.

Device pipeline (PE does ALL the streaming compute; measured fp8
LDWEIGHTS+matmul cadence ~27-32 ns per 128-col chunk, far above the DMA
rate, so the kernel sits on the DMA roofline):

  Host pre-transposes rows into a feature-major "2-block" fp8 layout
  (two train rows per column: features on partitions 0-63 / 64-127).
  Each 128-col chunk is one fp8 LDWEIGHTS (fast-weight-load) + one
  [128,2] matmul against a block-selector holding -2*q; PSUM
  accumulates -2 x~.q for 256 rows per chunk, 489 chunks = the whole
  shard, into a persistent 2-bank PSUM accumulator. Supertiles
  alternate between the two HWDGE rings (nc.sync / nc.scalar), ramping
  4->64 chunks on both rings so steady-state transfers are 1 MB. All
  dma_starts are emitted before any drain compute so no descriptor
  generation ever queues behind a data-dependent op.

  Drain, split so the first ~half runs mid-stream on the idle DVE/ACT
  engines: one fused DVE scalar_tensor_tensor gives d2n = -s*u8 - ps
  (= C - d^2, a monotone image of the weights), so the per-partition
  top-8 runs directly on d2n with NO activation in the selection path.
  The Z chain needs w = exp(-sqrt(d^2)); sqrt is computed as
  exp(0.5*ln(.)) so that ALL activations (Ln, Exp, Exp) come from the
  single natural_log_exp table set -- avoiding the 1.28-us table reload
  that alternating Sqrt/Exp costs (no act set contains both). At the
  tail the ACT chain for the second half overlaps the DVE top-8 scan.
  vals/idx for both halves and the Z-partial pack into one [128, 33]
  u32 tensor -> one output DMA.

Host merge: global top-T among per-partition top-8x2 candidates (by
d2n, same order as w) -> exact re-rank -> top-32 exact weights /
device-summed Z -> 32-row gather from y_train (y_train never touches
the device).
"""

import numpy as np

N = 1_000_000
D = 64
K = 32
NCORES = 8
SHARD = N // NCORES            # 125000 rows per core
PROWS = 128                    # SBUF partitions

CHUNK_ROWS = 256               # rows per PE chunk (2 blocks x 128)
NCHUNK = 489                   # ceil(125000 / 256) -> 184 pad rows
PE_ROWS = NCHUNK * CHUNK_ROWS  # 125184
D2COLS = 2 * NCHUNK            # 978 distance columns per partition
CAND = 8                       # top-8 per partition per half
TOPT = 192                     # host-side exact re-rank pool

PE_ST_SIZES = [8, 8, 32, 32, 64, 64, 64, 64, 64, 56, 33]
assert sum(PE_ST_SIZES) == NCHUNK
PE_MAX_ST = max(PE_ST_SIZES)
# Drain segments: [0, SEG1) after supertile SPLIT1_ST, [SEG1, SEG2) after
# supertile SPLIT2_ST, [SEG2, D2COLS) at the end. Each segment's PSUM
# reads are emitted right after the matmuls covering it, so the
# program-order dependency set stays minimal and the first two segments
# drain while the stream continues.
SPLIT1_ST = 7
SPLIT2_ST = 9
SEG1 = 2 * sum(PE_ST_SIZES[: SPLIT1_ST + 1])     # 672
SEG2 = 2 * sum(PE_ST_SIZES[: SPLIT2_ST + 1])     # 912

_CACHE = {}
LAST_RESULTS = None            # BassKernelResults of the most recent run


def _act_override():
    """Steer Bacc's activation-table-set picker so Ln and Exp both
    resolve to the one set that contains them both
    (natural_log_exp_and_others). The default first-match pick puts Ln
    and Exp in different sets, and every Ln->Exp transition then costs a
    1.28 us table reload on the ACT critical path. The walrus backend
    validates the chosen set id against the same stock act_info.json, so
    only Bacc's view (the input to insert_act_table_loads) needs
    narrowing. No-op if the combined set is absent."""
    import functools

    import concourse.bacc as bacc
    from concourse import mybir

    if getattr(_act_override, "done", False):
        return
    orig = bacc.get_activation_tables
    both = {mybir.ActivationFunctionType.Ln, mybir.ActivationFunctionType.Exp}

    @functools.cache
    def narrowed(arch):
        t = dict(orig(arch))
        if "natural_log_exp_and_others" in t and both <= t[
            "natural_log_exp_and_others"
        ]:
            for name in t:
                if name != "natural_log_exp_and_others":
                    t[name] = t[name] - both
        return t

    bacc.get_activation_tables = narrowed
    _act_override.done = True


def _build_nc():
    import concourse.bacc as bacc
    import concourse.tile as tile
    from concourse import mybir

    f32 = mybir.dt.float32
    fp8 = mybir.dt.float8e4
    u8 = mybir.dt.uint8
    u32 = mybir.dt.uint32
    Act = mybir.ActivationFunctionType
    Alu = mybir.AluOpType

    _act_override()
    nc = bacc.Bacc("TRN2", target_bir_lowering=False, debug=False)
    xt2_d = nc.dram_tensor(
        "xt2", [PROWS, NCHUNK * PROWS], fp8, kind="ExternalInput"
    ).ap()
    nx_d = nc.dram_tensor("nx", [PROWS, D2COLS], u8, kind="ExternalInput").ap()
    sel_d = nc.dram_tensor("sel", [PROWS, 2], fp8, kind="ExternalInput").ap()
    sc_d = nc.dram_tensor("sc", [PROWS, 2], f32, kind="ExternalInput").ap()
    out_d = nc.dram_tensor("pack", [PROWS, 49], u32, kind="ExternalOutput").ap()

    SEGS = [(0, SEG1), (SEG1, SEG2), (SEG2, D2COLS)]

    with tile.TileContext(nc) as tc:
        with (
            tc.tile_pool(name="persist", bufs=1) as persist,
            tc.tile_pool(name="xs", bufs=8) as xs_pool,
            tc.tile_pool(name="psum", bufs=1, space="PSUM") as psum_pool,
        ):
            sel = persist.tile([PROWS, 2], fp8)
            nc.sync.dma_start(out=sel[:], in_=sel_d[:])
            sc = persist.tile([PROWS, 2], f32)
            nc.scalar.dma_start(out=sc[:], in_=sc_d[:])
            nsvec = sc[:, 0:1]                     # -s (negated norm scale)
            cvec = sc[:, 1:2]                      # C = norm offset (+||q||^2)
            nx = persist.tile([PROWS, D2COLS], u8)

            d2n = persist.tile([PROWS, D2COLS], f32)
            wz = persist.tile([PROWS, D2COLS], f32)
            pack = persist.tile([PROWS, 49], u32)
            zps = persist.tile([PROWS, 3], f32)

            # Persistent PSUM accumulator: 978 f32 columns = 2 banks; no
            # mid-stream drain so PE streams matmuls back-to-back.
            ps = psum_pool.tile([PROWS, D2COLS], f32)

            def dve_scan(si):
                # d2n = C - d^2 (monotone in w): fused -s*u8norm - ps,
                # then the per-partition top-8 straight off d2n. Emitted
                # right after the matmuls covering [lo, hi) so the
                # program-order PSUM dep set stays minimal. DVE-only: the
                # DVE queue hosts no DMA ring, so mid-loop emission cannot
                # stall descriptor generation.
                lo, hi = SEGS[si]
                seg = slice(lo, hi)
                vals_ap = pack[:, 16 * si : 16 * si + 8].bitcast(f32)
                idx_ap = pack[:, 16 * si + 8 : 16 * si + 16]
                nc.vector.scalar_tensor_tensor(
                    d2n[:, seg], nx[:, seg], nsvec, ps[:, seg],
                    Alu.mult, Alu.subtract,
                )
                nc.vector.max(out=vals_ap, in_=d2n[:, seg])
                nc.vector.max_index(out=idx_ap, in_max=vals_ap,
                                    in_values=d2n[:, seg])

            def act_z(si):
                # Z chain (ACT): t=ln(d^2); d=exp(t/2); w=exp(-d) with
                # per-partition accum -- all from the single ln+exp table
                # set. Emitted after every dma_start (the ACT queue doubles
                # as a DMA ring; a data-gated op ahead of a DGE would stall
                # the ring) and execution is gated on d2n via data deps.
                # Writes wz, separate from d2n, so ACT never WARs with the
                # DVE scans.
                lo, hi = SEGS[si]
                seg = slice(lo, hi)
                nc.scalar.activation(wz[:, seg], d2n[:, seg], Act.Ln,
                                     scale=-1.0, bias=cvec)
                nc.scalar.activation(wz[:, seg], wz[:, seg], Act.Exp,
                                     scale=0.5)
                nc.scalar.activation(wz[:, seg], wz[:, seg], Act.Exp,
                                     scale=-1.0, accum_out=zps[:, si : si + 1])

            # --- stream: all DMA issues + matmuls; the only other
            # emissions are DVE/ACT ops (no queue overlap with the rings'
            # descriptor generation, which all precedes them per queue) ---
            pe_done = 0
            for sti, g in enumerate(PE_ST_SIZES):
                fd = g * PROWS
                xs = xs_pool.tile([PROWS, PE_MAX_ST * PROWS], fp8, tag="xs")
                ring = nc.sync if sti % 2 == 0 else nc.scalar
                ring.dma_start(
                    out=xs[:, :fd],
                    in_=xt2_d[:, pe_done * PROWS : pe_done * PROWS + fd],
                )
                if sti == 1:
                    # (The ln+exp act-table load itself needs no warm-up
                    # op: the compiler-placed ATL before the first Ln has
                    # no data deps and executes as soon as the ACT queue
                    # reaches it, mid-stream.)
                    nc.sync.dma_start(out=nx[:], in_=nx_d[:])
                for j in range(g):
                    c = 2 * (pe_done + j)
                    nc.tensor.matmul(
                        out=ps[:, c : c + 2],
                        lhsT=xs[:, j * PROWS : (j + 1) * PROWS],
                        rhs=sel[:],
                        start=True,
                        stop=True,
                    )
                pe_done += g
                if sti == SPLIT1_ST:
                    dve_scan(0)
                elif sti == SPLIT2_ST:
                    dve_scan(1)

            # --- tail: only the last 2*33 columns' scan remains, plus the
            # (mostly mid-stream, data-gated) ACT Z chains ---
            dve_scan(2)
            act_z(0)
            act_z(1)
            act_z(2)
            zpo = pack[:, 48:49].bitcast(f32)
            nc.vector.tensor_add(zpo, zps[:, 0:1], zps[:, 1:2])
            nc.vector.tensor_add(zpo, zpo, zps[:, 2:3])
            nc.sync.dma_start(out=out_d[:], in_=pack[:])

    nc.compile()
    return nc


def _pe_layout(xc8):
    """[PE_ROWS, D] fp8 rows -> feature-major 2-block layout.

    xt2[b*64+f, j*128+m] = xc8[j*256 + b*128 + m, f]
    """
    r = xc8.reshape(NCHUNK, 2, PROWS, D)         # [j, b, m, f]
    return np.ascontiguousarray(
        r.transpose(1, 3, 0, 2).reshape(PROWS, NCHUNK * PROWS)
    )


def kernel(X_train, y_train, X_missing):
    import os

    import ml_dtypes

    from concourse.bass_utils import run_bass_kernel_spmd

    global LAST_RESULTS

    fp8 = ml_dtypes.float8_e4m3

    X_train = np.ascontiguousarray(np.asarray(X_train, dtype=np.float32))
    y_train = np.asarray(y_train, dtype=np.float32)
    q = np.asarray(X_missing, dtype=np.float32)

    if "nc" not in _CACHE:
        _CACHE["nc"] = _build_nc()
    nc = _CACHE["nc"]

    x8 = X_train.astype(fp8)                      # quantized rows
    x8f = x8.astype(np.float32)
    norms = np.einsum("ij,ij->i", x8f, x8f) + float(
        (q.astype(np.float64) ** 2).sum()
    )
    lo = float(norms.min())
    hi = float(norms.max())
    s = (hi - lo) / 254.0

    m2q8 = (-2.0 * q).astype(fp8)
    sel = np.zeros((PROWS, 2), fp8)
    sel[:D, 0] = m2q8
    sel[D:, 1] = m2q8
    sc = np.empty((PROWS, 2), np.float32)
    sc[:, 0] = -s
    sc[:, 1] = lo
    nq8 = np.round((norms - lo) / s)              # in [0, 254]

    in_maps = []
    for c in range(NCORES):
        lo_r = c * SHARD
        xc8 = np.zeros((PE_ROWS, D), fp8)
        xc8[:SHARD] = x8[lo_r : lo_r + SHARD]

        # nx[p, 2j+b] = u8norm[j*256 + b*128 + p]  (255 for pad rows)
        nrm = np.full(PE_ROWS, 255.0, np.float32)
        nrm[:SHARD] = nq8[lo_r : lo_r + SHARD]
        nx = np.ascontiguousarray(
            nrm.reshape(NCHUNK, 2, PROWS).transpose(2, 0, 1)
            .reshape(PROWS, D2COLS).astype(np.uint8)
        )

        in_maps.append({"xt2": _pe_layout(xc8), "nx": nx, "sel": sel, "sc": sc})

    trace = bool(int(os.environ.get("KNN_TRACE", "0")))
    res = run_bass_kernel_spmd(
        nc, in_maps, core_ids=list(range(NCORES)), trace=trace
    )
    LAST_RESULTS = res

    # Host-side merge: global softmax denominator + approximate top-TOPT
    # among per-partition top-8x3 candidates (ranked by d2n = C - d^2,
    # monotone in w), exact re-rank, weighted sum.
    z_total = 0.0
    all_vals = []
    all_rows = []
    p = np.arange(PROWS, dtype=np.int64)[:, None]
    for c in range(NCORES):
        packed = res.results[c]["pack"]
        z_total += float(packed[:, 48].view(np.float32).astype(np.float64).sum())
        vals = np.concatenate(
            [packed[:, 16 * s : 16 * s + 8].view(np.float32) for s in range(3)],
            axis=1,
        )
        idx = np.concatenate(
            [packed[:, 16 * s + 8 : 16 * s + 16].astype(np.int64) + off
             for s, off in enumerate((0, SEG1, SEG2))],
            axis=1,
        )
        local_row = (idx // 2) * CHUNK_ROWS + (idx % 2) * PROWS + p
        keep = local_row < SHARD
        all_vals.append(vals[keep].astype(np.float64))
        all_rows.append((c * SHARD + local_row)[keep])
    all_vals = np.concatenate(all_vals)
    all_rows = np.concatenate(all_rows)

    t = min(TOPT, len(all_vals))
    cand = np.argpartition(-all_vals, t - 1)[:t]
    rows = np.unique(all_rows[cand])
    diff = X_train[rows].astype(np.float64) - q.astype(np.float64)[None, :]
    d_exact = np.sqrt((diff * diff).sum(1))
    sel_k = np.argsort(d_exact)[:K]
    w = np.exp(-d_exact[sel_k]) / z_total
    out = (w[:, None] * y_train[rows[sel_k]].astype(np.float64)).sum(axis=0)
    return out[None, :].astype(np.float32)
